# revision 37
# baseline (speedup 1.0000x reference)
"""Trainium2 Bass kernel for nn_DecoderLayer (B=4,S=2048,D=1024,H=16,FF=4096).

Sharding: 16 heads / 8 cores = 2 heads per core (tensor/head parallel) for
attention; the two local heads are summed in SBUF and written once (bf16) to
a chunked DRAM buffer; two ReduceScatters ({b0,b1,b2} then {b3}) combine the
cores; token-parallel LN+FFN on each core's 1/8 token shard (4 chunks of 256
tokens); host concatenates the shards.

Numerics: bf16 everywhere on the PE (fast weight load), fp32 accumulation in
PSUM, softmax in fp32 on ACT/DVE, LayerNorms/residuals in fp32. The 1/sqrt(64)
score scale is folded into Wq/bq on the host. X^T is produced by DMA-transpose
(X-bar) from a host-provided bf16 copy of the input.

o_ext layout: per batch 8 chunks of 257 rows (256 tokens + 1 vmean row) so a
ReduceScatter over 8 cores lands each core on whole chunks. Phase A covers
batches 0-2 (24 chunks -> 3 chunks/core), phase B batch 3 (8 -> 1 chunk/core).
"""
import numpy as np
import ml_dtypes
from contextlib import ExitStack

import concourse.bass as bass
import concourse.tile as tile
from concourse import bacc, mybir

dt = mybir.dt
F32 = dt.float32
BF16 = dt.bfloat16
AF = mybir.ActivationFunctionType
ALU = mybir.AluOpType
AX = mybir.AxisListType

KD = 64
EPS = 1e-5
NEG = -30000.0

CFG_MAIN = dict(B=4, S=2048, D=1024, FF=4096, ncores=8, HPC=2)


def build_nc(B, S, D, FF, ncores, HPC):
    DC = D // 128          # d chunks (8)
    TB = S // 128          # key blocks per batch (16)
    IC = S // 256          # query chunks (256 wide) per batch (8)
    EC = D // 512          # 512-wide e chunks (2)
    TOKC = S // 512        # 512-token chunks (4)
    FB = FF // 128         # ff blocks (32)
    CPB = S // 256         # 256-token chunks per batch (8)
    shard = B * S // ncores
    SB = shard // 128      # token blocks per core shard (8)
    BA = 3                 # batches in RS phase A
    ROWA = BA * CPB * 257  # 6168
    ROWB = CPB * 257       # 2056
    GF = 4                 # fb group size for y accumulation
    NPAIR = 2              # FFN processes 2 pairs of 2 chunks
    TPP = 512              # tokens per pair
    BPP = 4                # 128-blocks per pair

    nc = bacc.Bacc("TRN2", target_bir_lowering=False, debug=False,
                   enable_asserts=False, num_devices=ncores)

    # ---- DRAM I/O ----
    xb_d = nc.dram_tensor("xb", [B * S, D], BF16, kind="ExternalInput").ap()
    xs_d = nc.dram_tensor("xs", [shard, D], F32, kind="ExternalInput").ap()
    wq_d = nc.dram_tensor("wq", [HPC, D, KD], BF16, kind="ExternalInput").ap()
    wk_d = nc.dram_tensor("wk", [HPC, D, KD], BF16, kind="ExternalInput").ap()
    wv_d = nc.dram_tensor("wv", [HPC, D, D], BF16, kind="ExternalInput").ap()
    bqkc_d = nc.dram_tensor("bqkc", [128, 2], F32, kind="ExternalInput").ap()
    padb_d = nc.dram_tensor("padb", [B, 128, TB], F32, kind="ExternalInput").ap()
    q1m_d = nc.dram_tensor("q1m", [128, SB], F32, kind="ExternalInput").ap()
    qp_d = nc.dram_tensor("qp", [128, SB], F32, kind="ExternalInput").ap()
    cm_d = nc.dram_tensor("cmask", [128, 128], F32, kind="ExternalInput").ap()
    id_d = nc.dram_tensor("ident", [128, 128], F32, kind="ExternalInput").ap()
    w1_d = nc.dram_tensor("w1s", [FB, 128, D], BF16, kind="ExternalInput").ap()
    w2_d = nc.dram_tensor("w2", [FF, D], BF16, kind="ExternalInput").ap()
    b1t_d = nc.dram_tensor("b1t", [128, FB], F32, kind="ExternalInput").ap()
    b2b_d = nc.dram_tensor("b2b", [128, D], F32, kind="ExternalInput").ap()
    ln1w_d = nc.dram_tensor("ln1w", [128, D], F32, kind="ExternalInput").ap()
    ln1b_d = nc.dram_tensor("ln1b", [128, D], F32, kind="ExternalInput").ap()
    ln2w_d = nc.dram_tensor("ln2w", [128, D], F32, kind="ExternalInput").ap()
    ln2b_d = nc.dram_tensor("ln2b", [128, D], F32, kind="ExternalInput").ap()
    out_d = nc.dram_tensor("out", [shard, D], F32, kind="ExternalOutput").ap()

    with tile.TileContext(nc) as tc, ExitStack() as ctx0:
        pbig = ctx0.enter_context(tc.tile_pool(name="pbig", bufs=2, space="PSUM"))
        pmed = ctx0.enter_context(tc.tile_pool(name="pmed", bufs=2, space="PSUM"))
        pdn = ctx0.enter_context(tc.tile_pool(name="pdn", bufs=2, space="PSUM"))
        dramp = ctx0.enter_context(tc.tile_pool(name="dram", bufs=1, space="DRAM"))
        consts = ctx0.enter_context(tc.tile_pool(name="const", bufs=1))
        smalls = ctx0.enter_context(tc.tile_pool(name="smalls", bufs=6))

        o_ext_a = dramp.tile([ROWA, D], BF16, name="oexta")
        o_ext_b = dramp.tile([ROWB, D], BF16, name="oextb")
        rs_a = dramp.tile([ROWA // ncores, D], BF16, name="rsouta")
        rs_b = dramp.tile([ROWB // ncores, D], BF16, name="rsoutb")

        # ---- constants ----
        ident = consts.tile([128, 128], F32, tag="ident")
        nc.sync.dma_start(ident[:], id_d[:])
        cmask = consts.tile([128, 128], F32, tag="cmask")
        nc.sync.dma_start(cmask[:], cm_d[:])
        onesb = consts.tile([1, 512], BF16, tag="onesb")
        nc.vector.memset(onesb[:], 1.0)
        onescol = consts.tile([128, 1], BF16, tag="onescol")
        nc.vector.memset(onescol[:], 1.0)

        # =================== attention phase ===================
        with ExitStack() as actx:
            xtp = actx.enter_context(tc.tile_pool(name="xt", bufs=1))
            wvp = actx.enter_context(tc.tile_pool(name="wv", bufs=1))
            vp = actx.enter_context(tc.tile_pool(name="v", bufs=1))
            qkp = actx.enter_context(tc.tile_pool(name="qkt", bufs=2))
            ep = actx.enter_context(tc.tile_pool(name="e", bufs=6))
            osb0p = actx.enter_context(tc.tile_pool(name="osb0", bufs=16))
            osb1p = actx.enter_context(tc.tile_pool(name="osb1", bufs=3))
            vrp = actx.enter_context(tc.tile_pool(name="vr", bufs=2))
            padp = actx.enter_context(tc.tile_pool(name="pad", bufs=2))
            aconp = actx.enter_context(tc.tile_pool(name="acon", bufs=1))

            # layout: [q_h0 | q_h1 | k_h0 | k_h1], 64 cols each
            wqk = aconp.tile([128, DC * 2 * HPC * KD], BF16, tag="wqk")
            wqk3 = wqk[:].rearrange("p (c r) -> p c r", r=2 * HPC * KD)
            for h in range(HPC):
                for w in range(2):
                    g = w * HPC * KD + h * KD
                    nc.sync.dma_start(
                        wqk3[:, :, g:g + KD],
                        (wq_d if w == 0 else wk_d)[h].rearrange(
                            "(c p) k -> p c k", p=128))
            bqkc = aconp.tile([128, 2], F32, tag="bqkc")
            nc.sync.dma_start(bqkc[:], bqkc_d[:])
            def load_xt(b):
                # X_b^T via DMA transpose (bf16)
                xt = xtp.tile([128, DC * S], BF16, tag="xt", name=f"xt{b}")
                for c in range(DC):
                    nc.sync.dma_start(
                        xt[:, c * S:(c + 1) * S],
                        xb_d[b * S:(b + 1) * S, c * 128:(c + 1) * 128],
                        transpose=True)
                return xt

            # prefetch batch 0's X^T before the (late-needed) V weights
            xt0 = load_xt(0)

            wvs = []
            for h in range(HPC):
                wv1 = aconp.tile([128, DC * D], BF16, tag=f"wvs{h}",
                                 name=f"wvs{h}")
                nc.sync.dma_start(
                    wv1[:].rearrange("p (c e) -> p c e", c=DC),
                    wv_d[h].rearrange("(c p) e -> p c e", p=128))
                wvs.append(wv1)

            for b in range(B):
                xt = xt0 if b == 0 else load_xt(b)
                xsum = smalls.tile([128, DC], F32, tag="xsum")
                for c in range(DC):
                    nc.vector.tensor_reduce(
                        xsum[:, c:c + 1], xt[:, c * S:(c + 1) * S],
                        AX.X, ALU.add)
                xsumb = smalls.tile([128, DC], BF16, tag="xsumb")
                nc.scalar.copy(xsumb[:], xsum[:])

                padt = padp.tile([128, TB], F32, tag="pad")
                nc.sync.dma_start(padt[:], padb_d[b])

                # ---- packed Q/K projection for BOTH heads: out partitions
                # 0:64 = head 0, 64:128 = head 1 (q pre-scaled on host) ----
                qt = qkp.tile([128, S], BF16, tag="qt", name="qt")
                kt = qkp.tile([128, S], BF16, tag="kt", name="kt")
                for w, dst in ((0, qt), (1, kt)):
                    g = w * HPC * KD
                    for c4 in range(TOKC):
                        ps = pmed.tile([128, 512], F32, tag="med")
                        for c in range(DC):
                            nc.tensor.matmul(
                                ps[:], wqk3[:, c, g:g + 128],
                                xt[:, c * S + c4 * 512:c * S + c4 * 512 + 512],
                                start=(c == 0), stop=(c == DC - 1))
                        nc.vector.tensor_scalar_add(
                            dst[:, c4 * 512:c4 * 512 + 512], ps[:],
                            bqkc[:, w:w + 1])

                osbs = {}
                vrow_h = [None, None]
                for h in range(HPC):
                    # ---- V projection, bias-free: softmax rows sum to 1 so
                    # the per-head +bv folds into a constant Σbv/H the host
                    # adds to the residual ----
                    v = vp.tile([128, TB * D], BF16, tag="v")
                    for tb in range(TB):
                        vps = pbig.tile([128, D], F32, tag="big")
                        for c in range(DC):
                            for ec in range(EC):
                                nc.tensor.matmul(
                                    vps[:, ec * 512:ec * 512 + 512],
                                    xt[:, c * S + tb * 128:c * S + tb * 128 + 128],
                                    wvs[h][:, c * D + ec * 512:c * D + ec * 512 + 512],
                                    start=(c == 0), stop=(c == DC - 1))
                        nc.scalar.copy(v[:, tb * D:tb * D + D], vps[:])

                    # ---- vmean row (for fully-padded queries), via xsum ----
                    vmps = [pmed.tile([1, 512], F32, tag="med",
                                      name=f"vmps{ec}")
                            for ec in range(EC)]
                    for ec in range(EC):
                        for c in range(DC):
                            nc.tensor.matmul(
                                vmps[ec][:], xsumb[:, c:c + 1],
                                wvs[h][:, c * D + ec * 512:c * D + ec * 512 + 512],
                                start=(c == 0), stop=(c == DC - 1))
                    vrow = vrp.tile([1, D], F32, tag="vrow")
                    for ec in range(EC):
                        nc.vector.tensor_scalar_mul(
                            vrow[0:1, ec * 512:ec * 512 + 512], vmps[ec][:],
                            1.0 / S)
                    vrow_h[h] = vrow
                    if h == 1:
                        nc.vector.tensor_add(vrow[:], vrow_h[0][:], vrow[:])
                        vsb = vrp.tile([1, D], BF16, tag="vsb")
                        nc.scalar.copy(vsb[:], vrow[:])
                        for k in range(CPB):
                            if b < BA:
                                vr = (b * CPB + k) * 257 + 256
                                nc.sync.dma_start(
                                    o_ext_a[vr:vr + 1, :], vsb[:])
                            else:
                                vr = k * 257 + 256
                                nc.sync.dma_start(
                                    o_ext_b[vr:vr + 1, :], vsb[:])

                    # ---- attention for this head ----
                    qv = qt[64 * h:64 * h + 64, :]
                    kv = kt[64 * h:64 * h + 64, :]
                    for ic in range(IC):
                        ops = [pbig.tile([128, D], F32, tag="big",
                                         name=f"ops{s}")
                               for s in range(2)]
                        dnp = [pdn.tile([128, 1], F32, tag="dn",
                                        name=f"dnp{s}")
                               for s in range(2)]
                        ntb = 2 * ic + 2

                        def mk_e(tb):
                            st = pmed.tile([128, 256], F32, tag="med",
                                           name=f"st{tb}")
                            nc.tensor.matmul(
                                st[:], kv[:, tb * 128:tb * 128 + 128],
                                qv[:, ic * 256:ic * 256 + 256],
                                start=True, stop=True)
                            if tb >= 2 * ic:
                                off = (tb - 2 * ic) * 128
                                nc.vector.tensor_add(
                                    st[:, off:off + 128],
                                    st[:, off:off + 128], cmask[:])
                            e = ep.tile([128, 256], BF16, tag="e",
                                        name=f"e{tb}")
                            nc.scalar.activation(e[:], st[:], AF.Exp,
                                                 bias=padt[:, tb:tb + 1],
                                                 scale=1.0)
                            return e

                        epipe = {0: mk_e(0)}
                        if ntb > 1:
                            epipe[1] = mk_e(1)
                        for tb in range(ntb):
                            e = epipe.pop(tb)
                            if tb + 2 < ntb:
                                epipe[tb + 2] = mk_e(tb + 2)
                            for s in range(2):
                                ib = 2 * ic + s
                                if tb > ib:
                                    continue
                                for ec in range(EC):
                                    nc.tensor.matmul(
                                        ops[s][:, ec * 512:ec * 512 + 512],
                                        e[:, s * 128:s * 128 + 128],
                                        v[:, tb * D + ec * 512:
                                          tb * D + ec * 512 + 512],
                                        start=(tb == 0), stop=(tb == ib))
                                nc.tensor.matmul(
                                    dnp[s][:],
                                    e[:, s * 128:s * 128 + 128], onescol[:],
                                    start=(tb == 0), stop=(tb == ib))
                        for s in range(2):
                            dsb = smalls.tile([128, 1], F32, tag="dsb")
                            nc.vector.tensor_scalar_add(dsb[:], dnp[s][:],
                                                        1e-30)
                            rd = smalls.tile([128, 1], F32, tag="rd")
                            nc.vector.reciprocal(rd[:], dsb[:])
                            if h == 0:
                                o0 = osb0p.tile([128, D], BF16, tag="osb0")
                                if s == 0:
                                    nc.scalar.activation(o0[:], ops[s][:],
                                                         AF.Copy, scale=rd[:])
                                else:
                                    nc.vector.tensor_scalar_mul(
                                        o0[:], ops[s][:], rd[:])
                                osbs[(ic, s)] = o0
                            else:
                                o1 = osb1p.tile([128, D], BF16, tag="osb1")
                                if s == 0:
                                    nc.scalar.activation(o1[:], ops[s][:],
                                                         AF.Copy, scale=rd[:])
                                else:
                                    nc.vector.tensor_scalar_mul(
                                        o1[:], ops[s][:], rd[:])
                                o0 = osbs.pop((ic, s))
                                nc.vector.tensor_add(o0[:], o0[:], o1[:])
                                if b < BA:
                                    r0 = (b * CPB + ic) * 257 + s * 128
                                    nc.sync.dma_start(
                                        o_ext_a[r0:r0 + 128, :], o0[:])
                                else:
                                    r0 = ic * 257 + s * 128
                                    nc.sync.dma_start(
                                        o_ext_b[r0:r0 + 128, :], o0[:])

                if b == BA - 1:
                    nc.gpsimd.collective_compute(
                        "ReduceScatter", ALU.add,
                        replica_groups=[list(range(ncores))],
                        ins=[o_ext_a.opt()], outs=[rs_a.opt()])
                if b == B - 1:
                    nc.gpsimd.collective_compute(
                        "ReduceScatter", ALU.add,
                        replica_groups=[list(range(ncores))],
                        ins=[o_ext_b.opt()], outs=[rs_b.opt()])

        # =================== LN + FFN on the token shard ===================
        with ExitStack() as fctx:
            fcon = fctx.enter_context(tc.tile_pool(name="fcon", bufs=1))
            stg = fctx.enter_context(tc.tile_pool(name="stg", bufs=5))
            x1p = fctx.enter_context(tc.tile_pool(name="x1", bufs=2 * BPP + 1))
            x1tp = fctx.enter_context(tc.tile_pool(name="x1t", bufs=1))
            htp = fctx.enter_context(tc.tile_pool(name="hts", bufs=1))
            w1p = fctx.enter_context(tc.tile_pool(name="w1s", bufs=6))
            w2p = fctx.enter_context(tc.tile_pool(name="w2s", bufs=GF + 1))
            ysp = fctx.enter_context(tc.tile_pool(name="ysb", bufs=2 * BPP - 1))
            vtp = fctx.enter_context(tc.tile_pool(name="vt", bufs=2))
            outp = fctx.enter_context(tc.tile_pool(name="outp", bufs=2))

            b1t = fcon.tile([128, FB], F32, tag="b1t")
            nc.sync.dma_start(b1t[:], b1t_d[:])
            b2b = fcon.tile([128, D], F32, tag="b2b")
            nc.sync.dma_start(b2b[:], b2b_d[:])
            lnw = []
            for nm, dd in [("ln1w", ln1w_d), ("ln1b", ln1b_d),
                           ("ln2w", ln2w_d), ("ln2b", ln2b_d)]:
                t = fcon.tile([128, D], F32, tag=nm)
                nc.sync.dma_start(t[:], dd[:])
                lnw.append(t)
            ln1w, ln1b, ln2w, ln2b = lnw
            q1m = fcon.tile([128, SB], F32, tag="q1m")
            nc.sync.dma_start(q1m[:], q1m_d[:])
            qp = fcon.tile([128, SB], F32, tag="qp")
            nc.sync.dma_start(qp[:], qp_d[:])

            def layer_norm(x_ap, w_ap, b_ap, out_ap):
                G = D // 512
                st6 = smalls.tile([128, 6 * G], F32, tag="st6")
                for gg in range(G):
                    nc.vector.bn_stats(st6[:, 6 * gg:6 * gg + 6],
                                       x_ap[:, 512 * gg:512 * gg + 512])
                mv = smalls.tile([128, 2], F32, tag="mv")
                nc.vector.bn_aggr(mv[:], st6[:])
                ve = smalls.tile([128, 1], F32, tag="ve")
                nc.vector.tensor_scalar_add(ve[:], mv[:, 1:2], EPS)
                sd = smalls.tile([128, 1], F32, tag="sd")
                nc.scalar.sqrt(sd[:], ve[:])
                rs_ = smalls.tile([128, 1], F32, tag="rs")
                nc.vector.reciprocal(rs_[:], sd[:])
                nc.vector.tensor_scalar_sub(x_ap, x_ap, mv[:, 0:1])
                nc.vector.scalar_tensor_tensor(out_ap, x_ap, rs_[:], w_ap,
                                               ALU.mult, ALU.mult)
                nc.vector.tensor_add(out_ap, out_ap, b_ap)

            for p in range(NPAIR):
                dma_q = nc.sync
                x1s = []
                for jj in range(2):
                    j = 2 * p + jj
                    if j < 3:
                        src, base = rs_a, j * 257
                    else:
                        src, base = rs_b, 0
                    # broadcast this chunk's vtot row across partitions
                    vtr = vtp.tile([1, D], BF16, tag="vtr")
                    dma_q.dma_start(vtr[:], src[base + 256:base + 257, :])
                    vtb = vtp.tile([128, D], F32, tag="vtb")
                    for ec in range(EC):
                        bps = pmed.tile([128, 512], F32, tag="med")
                        nc.tensor.matmul(bps[:], onesb[0:1, 0:128],
                                         vtr[0:1, ec * 512:ec * 512 + 512],
                                         start=True, stop=True)
                        nc.scalar.copy(vtb[:, ec * 512:ec * 512 + 512], bps[:])

                    # ---- x0 = blend(attn) + residual; x1 = LN1(x0) ----
                    for tl in range(2):
                        sblk = j * 2 + tl
                        rsb = stg.tile([128, D], BF16, tag="rsb", bufs=4)
                        dma_q.dma_start(
                            rsb[:], src[base + tl * 128:base + tl * 128 + 128, :])
                        xsb = stg.tile([128, D], F32, tag="stg")
                        dma_q.dma_start(
                            xsb[:], xs_d[sblk * 128:sblk * 128 + 128, :])
                        t0 = stg.tile([128, D], F32, tag="stg")
                        nc.vector.scalar_tensor_tensor(
                            t0[:], rsb[:], q1m[:, sblk:sblk + 1], xsb[:],
                            ALU.mult, ALU.add)
                        x0 = stg.tile([128, D], F32, tag="stg")
                        nc.vector.scalar_tensor_tensor(
                            x0[:], vtb[:], qp[:, sblk:sblk + 1], t0[:],
                            ALU.mult, ALU.add)
                        x1 = x1p.tile([128, D], F32, tag="x1")
                        layer_norm(x0[:], ln1w[:], ln1b[:], x1[:])
                        x1s.append(x1)

                # ---- x1^T (bf16) for the pair ----
                x1t = x1tp.tile([128, DC * TPP], BF16, tag="x1t")
                for q in range(BPP):
                    for c in range(DC):
                        tp = pmed.tile([128, 128], F32, tag="med")
                        nc.tensor.transpose(
                            tp[:], x1s[q][:, c * 128:c * 128 + 128], ident[:])
                        nc.scalar.copy(
                            x1t[:, c * TPP + q * 128:c * TPP + q * 128 + 128],
                            tp[:])

                # ---- hT = relu(W1^T x1^T + b1) (bf16) ----
                hts = htp.tile([128, FB * TPP], BF16, tag="hts")
                for fb in range(FB):
                    w1s = w1p.tile([128, D], BF16, tag="w1s")
                    dma_q.dma_start(w1s[:], w1_d[fb])
                    hps = pmed.tile([128, TPP], F32, tag="med")
                    for c in range(DC):
                        nc.tensor.matmul(hps[:], w1s[:, c * 128:c * 128 + 128],
                                         x1t[:, c * TPP:(c + 1) * TPP],
                                         start=(c == 0), stop=(c == DC - 1))
                    nc.scalar.activation(hts[:, fb * TPP:(fb + 1) * TPP],
                                         hps[:], AF.Relu,
                                         bias=b1t[:, fb:fb + 1], scale=1.0)

                # ---- y = hT.T @ W2 accumulated over fb groups ----
                ys_prev = [None] * BPP
                NG = FB // GF
                w2cache = {}
                for g in range(NG):
                    for q in range(BPP):
                        yps = pbig.tile([128, D], F32, tag="big")
                        for ji in range(GF):
                            fb = g * GF + ji
                            if q == 0:
                                w2s = w2p.tile([128, D], BF16, tag="w2s")
                                dma_q.dma_start(
                                    w2s[:], w2_d[fb * 128:fb * 128 + 128, :])
                                w2cache[fb] = w2s
                            w2s = w2cache[fb]
                            for ec in range(EC):
                                nc.tensor.matmul(
                                    yps[:, ec * 512:ec * 512 + 512],
                                    hts[:, fb * TPP + q * 128:
                                        fb * TPP + q * 128 + 128],
                                    w2s[:, ec * 512:ec * 512 + 512],
                                    start=(ji == 0), stop=(ji == GF - 1))
                        ysn = ysp.tile([128, D], F32, tag="ysb")
                        if g == 0:
                            nc.scalar.copy(ysn[:], yps[:])
                        else:
                            nc.vector.scalar_tensor_tensor(
                                ysn[:], yps[:], 1.0, ys_prev[q][:],
                                ALU.mult, ALU.add)
                        ys_prev[q] = ysn

                # ---- x2 = x1 + y + b2; out = LN2(x2) ----
                for q in range(BPP):
                    x2 = outp.tile([128, D], F32, tag="x2")
                    nc.vector.scalar_tensor_tensor(
                        x2[:], ys_prev[q][:], 1.0, x1s[q][:],
                        ALU.mult, ALU.add)
                    nc.vector.tensor_add(x2[:], x2[:], b2b[:])
                    ot = outp.tile([128, D], F32, tag="ot")
                    layer_norm(x2[:], ln2w[:], ln2b[:], ot[:])
                    row = (p * BPP + q) * 128
                    nc.sync.dma_start(out_d[row:row + 128, :], ot[:])

    nc.compile()
    return nc


# ------------------------- host side -------------------------

_NC_CACHE = {}


def _get_nc(cfg_key):
    if cfg_key not in _NC_CACHE:
        _NC_CACHE[cfg_key] = build_nc(**CFG_MAIN)
    return _NC_CACHE[cfg_key]


def core_chunks(c, B, S, ncores):
    """(batch, c0) chunk list owned by core c: 3 phase-A chunks + 1 phase-B."""
    CPB = S // 256
    out = [((3 * c + k) // CPB, (3 * c + k) % CPB) for k in range(3)]
    out.append((B - 1, c))
    return out


def make_in_maps(inputs, B, S, D, FF, ncores, HPC):
    """Build the per-core input dicts from the full (unsharded) inputs."""
    TB = S // 128
    shard = B * S // ncores
    SB = shard // 128
    FB = FF // 128
    H = ncores * HPC
    bf = ml_dtypes.bfloat16

    x = np.ascontiguousarray(
        np.asarray(inputs["input"], dtype=np.float32).reshape(B * S, D))
    xb = x.astype(bf)
    pad = np.asarray(inputs["padding_mask"], dtype=bool)
    Wq = np.asarray(inputs["Wq"], dtype=np.float32) * 0.125
    Wk = np.asarray(inputs["Wk"], dtype=np.float32)
    Wv = np.asarray(inputs["Wv"], dtype=np.float32)
    bq = np.asarray(inputs["bq"], dtype=np.float32) * 0.125
    bk = np.asarray(inputs["bk"], dtype=np.float32)
    bvv = np.asarray(inputs["bv"], dtype=np.float32)

    padb = np.where(pad, np.float32(NEG), np.float32(0.0))
    padb = np.ascontiguousarray(
        padb.reshape(B, TB, 128).transpose(0, 2, 1))

    cmask = np.zeros((128, 128), dtype=np.float32)
    cmask[np.tril_indices(128, -1)] = NEG

    w1 = np.asarray(inputs["ff1_w"], dtype=np.float32)
    w1s = np.ascontiguousarray(
        w1.reshape(D // 128, 128, FB, 128).transpose(2, 1, 0, 3)
        .reshape(FB, 128, D)).astype(bf)
    w2 = np.asarray(inputs["ff2_w"], dtype=np.float32).astype(bf)
    b1 = np.asarray(inputs["ff1_b"], dtype=np.float32)
    b1t = np.ascontiguousarray(b1.reshape(FB, 128).T)
    b2b = np.ascontiguousarray(
        np.broadcast_to(np.asarray(inputs["ff2_b"], np.float32), (128, D)))

    def bc(name):
        return np.ascontiguousarray(np.broadcast_to(
            np.asarray(inputs[name], np.float32), (128, D)))

    ident = np.eye(128, dtype=np.float32)
    padflat = pad.reshape(B * S)

    # softmax rows sum to 1, so every head's +bv contributes exactly bv_h to
    # the 16-head sum; fold sum(bv)/H into the residual (pad terms cancel).
    bvall = bvv.sum(axis=0).astype(np.float32) / H

    in_maps = []
    for c in range(ncores):
        h0 = c * HPC
        tok_idx = np.concatenate([
            bb * S + c0 * 256 + np.arange(256)
            for bb, c0 in core_chunks(c, B, S, ncores)])
        prow = padflat[tok_idx].reshape(SB, 128).T
        prow = prow.astype(np.float32)
        m = {
            "xb": xb,
            "xs": np.ascontiguousarray(x[tok_idx] + bvall[None, :]),
            "wq": np.ascontiguousarray(Wq[h0:h0 + HPC]).astype(bf),
            "wk": np.ascontiguousarray(Wk[h0:h0 + HPC]).astype(bf),
            "wv": np.ascontiguousarray(Wv[h0:h0 + HPC]).astype(bf),
            "bqkc": np.ascontiguousarray(np.stack(
                [np.concatenate([bq[h0], bq[h0 + 1]]),
                 np.concatenate([bk[h0], bk[h0 + 1]])], axis=1)
            ).astype(np.float32),
            "padb": padb,
            "q1m": np.ascontiguousarray((1.0 - prow) / H),
            "qp": np.ascontiguousarray(prow / H),
            "cmask": cmask,
            "ident": ident,
            "w1s": w1s,
            "w2": w2,
            "b1t": b1t,
            "b2b": b2b,
            "ln1w": bc("ln1_w"),
            "ln1b": bc("ln1_b"),
            "ln2w": bc("ln2_w"),
            "ln2b": bc("ln2_b"),
        }
        in_maps.append(m)
    return in_maps


def assemble_out(results, B, S, D, ncores):
    out = np.empty((B * S, D), dtype=np.float32)
    for c in range(ncores):
        r_ = np.asarray(results[c]["out"])
        for k, (bb, c0) in enumerate(core_chunks(c, B, S, ncores)):
            out[bb * S + c0 * 256: bb * S + c0 * 256 + 256] = \
                r_[k * 256:(k + 1) * 256]
    return out.reshape(B, S, D)


def kernel(**inputs):
    from concourse.bass_utils import run_bass_kernel_spmd
    cfg = CFG_MAIN
    B, S, D = cfg["B"], cfg["S"], cfg["D"]
    ncores = cfg["ncores"]
    nc = _get_nc("main")
    in_maps = make_in_maps(inputs, **cfg)
    res = run_bass_kernel_spmd(nc, in_maps, list(range(ncores)))
    return assemble_out(res.results, B, S, D, ncores).astype(np.float32)


# revision 44
# speedup vs baseline: 1.0291x; 1.0291x over previous
"""Trainium2 Bass kernel for nn_DecoderLayer (B=4,S=2048,D=1024,H=16,FF=4096).

Sharding: 16 heads / 8 cores = 2 heads per core (tensor/head parallel) for
attention; the two local heads are summed in SBUF and written once (bf16) to
a chunked DRAM buffer; two ReduceScatters ({b0,b1,b2} then {b3}) combine the
cores; token-parallel LN+FFN on each core's 1/8 token shard (4 chunks of 256
tokens); host concatenates the shards.

Numerics: bf16 everywhere on the PE (fast weight load), fp32 accumulation in
PSUM, softmax in fp32 on ACT/DVE, LayerNorms/residuals in fp32. The 1/sqrt(64)
score scale is folded into Wq/bq on the host. X^T is produced by DMA-transpose
(X-bar) from a host-provided bf16 copy of the input.

o_ext layout: per batch 8 chunks of 257 rows (256 tokens + 1 vmean row) so a
ReduceScatter over 8 cores lands each core on whole chunks. Phase A covers
batches 0-2 (24 chunks -> 3 chunks/core), phase B batch 3 (8 -> 1 chunk/core).
"""
import numpy as np
import ml_dtypes
from contextlib import ExitStack

import concourse.bass as bass
import concourse.tile as tile
from concourse import bacc, mybir

dt = mybir.dt
F32 = dt.float32
BF16 = dt.bfloat16
AF = mybir.ActivationFunctionType
ALU = mybir.AluOpType
AX = mybir.AxisListType

KD = 64
EPS = 1e-5
NEG = -30000.0

CFG_MAIN = dict(B=4, S=2048, D=1024, FF=4096, ncores=8, HPC=2)


def build_nc(B, S, D, FF, ncores, HPC):
    DC = D // 128          # d chunks (8)
    TB = S // 128          # key blocks per batch (16)
    IC = S // 256          # query chunks (256 wide) per batch (8)
    EC = D // 512          # 512-wide e chunks (2)
    TOKC = S // 512        # 512-token chunks (4)
    FB = FF // 128         # ff blocks (32)
    CPB = S // 256         # 256-token chunks per batch (8)
    shard = B * S // ncores
    SB = shard // 128      # token blocks per core shard (8)
    BA = 3                 # batches in RS phase A
    ROWA = BA * CPB * 257  # 6168
    ROWB = CPB * 257       # 2056
    GF = 4                 # fb group size for y accumulation
    NPAIR = 2              # FFN processes 2 pairs of 2 chunks
    TPP = 512              # tokens per pair
    BPP = 4                # 128-blocks per pair

    nc = bacc.Bacc("TRN2", target_bir_lowering=False, debug=False,
                   enable_asserts=False, num_devices=ncores)

    # ---- DRAM I/O ----
    xb_d = nc.dram_tensor("xb", [B * S, D], BF16, kind="ExternalInput").ap()
    xs_d = nc.dram_tensor("xs", [shard, D], F32, kind="ExternalInput").ap()
    wq_d = nc.dram_tensor("wq", [HPC, D, KD], BF16, kind="ExternalInput").ap()
    wk_d = nc.dram_tensor("wk", [HPC, D, KD], BF16, kind="ExternalInput").ap()
    wv_d = nc.dram_tensor("wv", [HPC, D, D], BF16, kind="ExternalInput").ap()
    bqkc_d = nc.dram_tensor("bqkc", [128, 2], F32, kind="ExternalInput").ap()
    padb_d = nc.dram_tensor("padb", [B, 128, TB], F32, kind="ExternalInput").ap()
    q1m_d = nc.dram_tensor("q1m", [128, SB], F32, kind="ExternalInput").ap()
    qp_d = nc.dram_tensor("qp", [128, SB], F32, kind="ExternalInput").ap()
    cm_d = nc.dram_tensor("cmask", [128, 128], F32, kind="ExternalInput").ap()
    id_d = nc.dram_tensor("ident", [128, 128], F32, kind="ExternalInput").ap()
    w1_d = nc.dram_tensor("w1s", [FB, 128, D], BF16, kind="ExternalInput").ap()
    w2_d = nc.dram_tensor("w2", [FF, D], BF16, kind="ExternalInput").ap()
    b1t_d = nc.dram_tensor("b1t", [128, FB], F32, kind="ExternalInput").ap()
    b2b_d = nc.dram_tensor("b2b", [128, D], F32, kind="ExternalInput").ap()
    ln1w_d = nc.dram_tensor("ln1w", [128, D], F32, kind="ExternalInput").ap()
    ln1b_d = nc.dram_tensor("ln1b", [128, D], F32, kind="ExternalInput").ap()
    ln2w_d = nc.dram_tensor("ln2w", [128, D], F32, kind="ExternalInput").ap()
    ln2b_d = nc.dram_tensor("ln2b", [128, D], F32, kind="ExternalInput").ap()
    out_d = nc.dram_tensor("out", [shard, D], F32, kind="ExternalOutput").ap()

    with tile.TileContext(nc) as tc, ExitStack() as ctx0:
        pbig = ctx0.enter_context(tc.tile_pool(name="pbig", bufs=2, space="PSUM"))
        pmed = ctx0.enter_context(tc.tile_pool(name="pmed", bufs=2, space="PSUM"))
        pdn = ctx0.enter_context(tc.tile_pool(name="pdn", bufs=2, space="PSUM"))
        dramp = ctx0.enter_context(tc.tile_pool(name="dram", bufs=1, space="DRAM"))
        consts = ctx0.enter_context(tc.tile_pool(name="const", bufs=1))
        smalls = ctx0.enter_context(tc.tile_pool(name="smalls", bufs=6))

        o_ext_a = dramp.tile([ROWA, D], BF16, name="oexta")
        o_ext_b = dramp.tile([ROWB, D], BF16, name="oextb")
        rs_a = dramp.tile([ROWA // ncores, D], BF16, name="rsouta")
        rs_b = dramp.tile([ROWB // ncores, D], BF16, name="rsoutb")

        # ---- constants ----
        ident = consts.tile([128, 128], F32, tag="ident")
        nc.sync.dma_start(ident[:], id_d[:])
        cmask = consts.tile([128, 128], F32, tag="cmask")
        nc.sync.dma_start(cmask[:], cm_d[:])
        onesb = consts.tile([1, 512], BF16, tag="onesb")
        nc.vector.memset(onesb[:], 1.0)
        onescol = consts.tile([128, 1], BF16, tag="onescol")
        nc.vector.memset(onescol[:], 1.0)

        # =================== attention phase ===================
        with ExitStack() as actx:
            xtp = actx.enter_context(tc.tile_pool(name="xt", bufs=1))
            wvp = actx.enter_context(tc.tile_pool(name="wv", bufs=1))
            vp = actx.enter_context(tc.tile_pool(name="v", bufs=1))
            qkp = actx.enter_context(tc.tile_pool(name="qkt", bufs=2))
            ep = actx.enter_context(tc.tile_pool(name="e", bufs=6))
            osb0p = actx.enter_context(tc.tile_pool(name="osb0", bufs=16))
            osb1p = actx.enter_context(tc.tile_pool(name="osb1", bufs=3))
            vrp = actx.enter_context(tc.tile_pool(name="vr", bufs=2))
            padp = actx.enter_context(tc.tile_pool(name="pad", bufs=2))
            aconp = actx.enter_context(tc.tile_pool(name="acon", bufs=1))

            # layout: [q_h0 | q_h1 | k_h0 | k_h1], 64 cols each
            wqk = aconp.tile([128, DC * 2 * HPC * KD], BF16, tag="wqk")
            wqk3 = wqk[:].rearrange("p (c r) -> p c r", r=2 * HPC * KD)
            for h in range(HPC):
                for w in range(2):
                    g = w * HPC * KD + h * KD
                    nc.sync.dma_start(
                        wqk3[:, :, g:g + KD],
                        (wq_d if w == 0 else wk_d)[h].rearrange(
                            "(c p) k -> p c k", p=128))
            bqkc = aconp.tile([128, 2], F32, tag="bqkc")
            nc.sync.dma_start(bqkc[:], bqkc_d[:])
            def load_xt(b):
                # X_b^T via DMA transpose (bf16)
                xt = xtp.tile([128, DC * S], BF16, tag="xt", name=f"xt{b}")
                for c in range(DC):
                    nc.sync.dma_start(
                        xt[:, c * S:(c + 1) * S],
                        xb_d[b * S:(b + 1) * S, c * 128:(c + 1) * 128],
                        transpose=True)
                return xt

            # prefetch batch 0's X^T before the (late-needed) V weights
            xt0 = load_xt(0)

            wvs = []
            for h in range(HPC):
                wv1 = aconp.tile([128, DC * D], BF16, tag=f"wvs{h}",
                                 name=f"wvs{h}")
                nc.sync.dma_start(
                    wv1[:].rearrange("p (c e) -> p c e", c=DC),
                    wv_d[h].rearrange("(c p) e -> p c e", p=128))
                wvs.append(wv1)

            for b in range(B):
                xt = xt0 if b == 0 else load_xt(b)
                xsum = smalls.tile([128, DC], F32, tag="xsum")
                for c in range(DC):
                    nc.vector.tensor_reduce(
                        xsum[:, c:c + 1], xt[:, c * S:(c + 1) * S],
                        AX.X, ALU.add)
                xsumb = smalls.tile([128, DC], BF16, tag="xsumb")
                nc.scalar.copy(xsumb[:], xsum[:])

                padt = padp.tile([128, TB], F32, tag="pad")
                nc.sync.dma_start(padt[:], padb_d[b])

                # ---- packed Q/K projection for BOTH heads: out partitions
                # 0:64 = head 0, 64:128 = head 1 (q pre-scaled on host) ----
                qt = qkp.tile([128, S], BF16, tag="qt", name="qt")
                kt = qkp.tile([128, S], BF16, tag="kt", name="kt")
                for w, dst in ((0, qt), (1, kt)):
                    g = w * HPC * KD
                    for c4 in range(TOKC):
                        ps = pmed.tile([128, 512], F32, tag="med")
                        for c in range(DC):
                            nc.tensor.matmul(
                                ps[:], wqk3[:, c, g:g + 128],
                                xt[:, c * S + c4 * 512:c * S + c4 * 512 + 512],
                                start=(c == 0), stop=(c == DC - 1))
                        nc.vector.tensor_scalar_add(
                            dst[:, c4 * 512:c4 * 512 + 512], ps[:],
                            bqkc[:, w:w + 1])

                osbs = {}
                vrow_h = [None, None]
                for h in range(HPC):
                    # ---- V projection, bias-free: softmax rows sum to 1 so
                    # the per-head +bv folds into a constant Σbv/H the host
                    # adds to the residual ----
                    v = vp.tile([128, TB * D], BF16, tag="v")
                    for tb in range(TB):
                        vps = pbig.tile([128, D], F32, tag="big")
                        for c in range(DC):
                            for ec in range(EC):
                                nc.tensor.matmul(
                                    vps[:, ec * 512:ec * 512 + 512],
                                    xt[:, c * S + tb * 128:c * S + tb * 128 + 128],
                                    wvs[h][:, c * D + ec * 512:c * D + ec * 512 + 512],
                                    start=(c == 0), stop=(c == DC - 1))
                        nc.scalar.copy(v[:, tb * D:tb * D + D], vps[:])

                    # ---- vmean row (for fully-padded queries), via xsum ----
                    vmps = [pmed.tile([1, 512], F32, tag="med",
                                      name=f"vmps{ec}")
                            for ec in range(EC)]
                    for ec in range(EC):
                        for c in range(DC):
                            nc.tensor.matmul(
                                vmps[ec][:], xsumb[:, c:c + 1],
                                wvs[h][:, c * D + ec * 512:c * D + ec * 512 + 512],
                                start=(c == 0), stop=(c == DC - 1))
                    vrow = vrp.tile([1, D], F32, tag="vrow")
                    for ec in range(EC):
                        nc.vector.tensor_scalar_mul(
                            vrow[0:1, ec * 512:ec * 512 + 512], vmps[ec][:],
                            1.0 / S)
                    vrow_h[h] = vrow
                    if h == 1:
                        nc.vector.tensor_add(vrow[:], vrow_h[0][:], vrow[:])
                        vsb = vrp.tile([1, D], BF16, tag="vsb")
                        nc.scalar.copy(vsb[:], vrow[:])
                        for k in range(CPB):
                            if b < BA:
                                vr = (b * CPB + k) * 257 + 256
                                nc.sync.dma_start(
                                    o_ext_a[vr:vr + 1, :], vsb[:])
                            else:
                                vr = k * 257 + 256
                                nc.sync.dma_start(
                                    o_ext_b[vr:vr + 1, :], vsb[:])

                    # ---- attention for this head ----
                    qv = qt[64 * h:64 * h + 64, :]
                    kv = kt[64 * h:64 * h + 64, :]
                    for ic in range(IC):
                        ops = [pbig.tile([128, D], F32, tag="big",
                                         name=f"ops{s}")
                               for s in range(2)]
                        dnp = [pdn.tile([128, 1], F32, tag="dn",
                                        name=f"dnp{s}")
                               for s in range(2)]
                        ntb = 2 * ic + 2

                        def mk_e(tb):
                            st = pmed.tile([128, 256], F32, tag="med",
                                           name=f"st{tb}")
                            nc.tensor.matmul(
                                st[:], kv[:, tb * 128:tb * 128 + 128],
                                qv[:, ic * 256:ic * 256 + 256],
                                start=True, stop=True)
                            if tb >= 2 * ic:
                                off = (tb - 2 * ic) * 128
                                nc.vector.tensor_add(
                                    st[:, off:off + 128],
                                    st[:, off:off + 128], cmask[:])
                            e = ep.tile([128, 256], BF16, tag="e",
                                        name=f"e{tb}")
                            nc.scalar.activation(e[:], st[:], AF.Exp,
                                                 bias=padt[:, tb:tb + 1],
                                                 scale=1.0)
                            return e

                        epipe = {0: mk_e(0)}
                        if ntb > 1:
                            epipe[1] = mk_e(1)
                        for tb in range(ntb):
                            e = epipe.pop(tb)
                            if tb + 2 < ntb:
                                epipe[tb + 2] = mk_e(tb + 2)
                            for s in range(2):
                                ib = 2 * ic + s
                                if tb > ib:
                                    continue
                                for ec in range(EC):
                                    nc.tensor.matmul(
                                        ops[s][:, ec * 512:ec * 512 + 512],
                                        e[:, s * 128:s * 128 + 128],
                                        v[:, tb * D + ec * 512:
                                          tb * D + ec * 512 + 512],
                                        start=(tb == 0), stop=(tb == ib))
                                nc.tensor.matmul(
                                    dnp[s][:],
                                    e[:, s * 128:s * 128 + 128], onescol[:],
                                    start=(tb == 0), stop=(tb == ib))
                        for s in range(2):
                            dsb = smalls.tile([128, 1], F32, tag="dsb")
                            nc.vector.tensor_scalar_add(dsb[:], dnp[s][:],
                                                        1e-30)
                            rd = smalls.tile([128, 1], F32, tag="rd")
                            nc.vector.reciprocal(rd[:], dsb[:])
                            if h == 0:
                                o0 = osb0p.tile([128, D], BF16, tag="osb0")
                                if s == 0:
                                    nc.scalar.activation(o0[:], ops[s][:],
                                                         AF.Copy, scale=rd[:])
                                else:
                                    nc.vector.tensor_scalar_mul(
                                        o0[:], ops[s][:], rd[:])
                                osbs[(ic, s)] = o0
                            else:
                                o1 = osb1p.tile([128, D], BF16, tag="osb1")
                                if s == 0:
                                    nc.scalar.activation(o1[:], ops[s][:],
                                                         AF.Copy, scale=rd[:])
                                else:
                                    nc.vector.tensor_scalar_mul(
                                        o1[:], ops[s][:], rd[:])
                                o0 = osbs.pop((ic, s))
                                nc.vector.tensor_add(o0[:], o0[:], o1[:])
                                if b < BA:
                                    r0 = (b * CPB + ic) * 257 + s * 128
                                    nc.sync.dma_start(
                                        o_ext_a[r0:r0 + 128, :], o0[:])
                                else:
                                    r0 = ic * 257 + s * 128
                                    nc.sync.dma_start(
                                        o_ext_b[r0:r0 + 128, :], o0[:])

                if b == BA - 1:
                    nc.gpsimd.collective_compute(
                        "ReduceScatter", ALU.add,
                        replica_groups=[list(range(ncores))],
                        ins=[o_ext_a.opt()], outs=[rs_a.opt()])
                if b == B - 1:
                    nc.gpsimd.collective_compute(
                        "ReduceScatter", ALU.add,
                        replica_groups=[list(range(ncores))],
                        ins=[o_ext_b.opt()], outs=[rs_b.opt()])

        # =================== LN + FFN on the token shard ===================
        with ExitStack() as fctx:
            fcon = fctx.enter_context(tc.tile_pool(name="fcon", bufs=1))
            stg = fctx.enter_context(tc.tile_pool(name="stg", bufs=5))
            x1p = fctx.enter_context(tc.tile_pool(name="x1", bufs=2 * BPP + 1))
            x1tp = fctx.enter_context(tc.tile_pool(name="x1t", bufs=1))
            htp = fctx.enter_context(tc.tile_pool(name="hts", bufs=1))
            w1p = fctx.enter_context(tc.tile_pool(name="w1s", bufs=6))
            w2p = fctx.enter_context(tc.tile_pool(name="w2s", bufs=GF + 1))
            ysp = fctx.enter_context(tc.tile_pool(name="ysb", bufs=2 * BPP - 1))
            vtp = fctx.enter_context(tc.tile_pool(name="vt", bufs=2))
            outp = fctx.enter_context(tc.tile_pool(name="outp", bufs=2))

            b1t = fcon.tile([128, FB], F32, tag="b1t")
            nc.sync.dma_start(b1t[:], b1t_d[:])
            b2b = fcon.tile([128, D], F32, tag="b2b")
            nc.sync.dma_start(b2b[:], b2b_d[:])
            lnw = []
            for nm, dd in [("ln1w", ln1w_d), ("ln1b", ln1b_d),
                           ("ln2w", ln2w_d), ("ln2b", ln2b_d)]:
                t = fcon.tile([128, D], F32, tag=nm)
                nc.sync.dma_start(t[:], dd[:])
                lnw.append(t)
            ln1w, ln1b, ln2w, ln2b = lnw
            q1m = fcon.tile([128, SB], F32, tag="q1m")
            nc.sync.dma_start(q1m[:], q1m_d[:])
            qp = fcon.tile([128, SB], F32, tag="qp")
            nc.sync.dma_start(qp[:], qp_d[:])

            def layer_norm(x_ap, w_ap, b_ap, out_ap):
                G = D // 512
                st6 = smalls.tile([128, 6 * G], F32, tag="st6")
                for gg in range(G):
                    nc.vector.bn_stats(st6[:, 6 * gg:6 * gg + 6],
                                       x_ap[:, 512 * gg:512 * gg + 512])
                mv = smalls.tile([128, 2], F32, tag="mv")
                nc.vector.bn_aggr(mv[:], st6[:])
                ve = smalls.tile([128, 1], F32, tag="ve")
                nc.vector.tensor_scalar_add(ve[:], mv[:, 1:2], EPS)
                sd = smalls.tile([128, 1], F32, tag="sd")
                nc.scalar.sqrt(sd[:], ve[:])
                rs_ = smalls.tile([128, 1], F32, tag="rs")
                nc.vector.reciprocal(rs_[:], sd[:])
                nc.vector.tensor_scalar_sub(x_ap, x_ap, mv[:, 0:1])
                nc.vector.scalar_tensor_tensor(out_ap, x_ap, rs_[:], w_ap,
                                               ALU.mult, ALU.mult)
                nc.vector.tensor_add(out_ap, out_ap, b_ap)

            for p in range(NPAIR):
                dma_q = nc.sync
                x1s = []
                for jj in range(2):
                    j = 2 * p + jj
                    if j < 3:
                        src, base = rs_a, j * 257
                    else:
                        src, base = rs_b, 0
                    # broadcast this chunk's vtot row across partitions
                    vtr = vtp.tile([1, D], BF16, tag="vtr")
                    dma_q.dma_start(vtr[:], src[base + 256:base + 257, :])
                    vtb = vtp.tile([128, D], F32, tag="vtb")
                    for ec in range(EC):
                        bps = pmed.tile([128, 512], F32, tag="med")
                        nc.tensor.matmul(bps[:], onesb[0:1, 0:128],
                                         vtr[0:1, ec * 512:ec * 512 + 512],
                                         start=True, stop=True)
                        nc.scalar.copy(vtb[:, ec * 512:ec * 512 + 512], bps[:])

                    # ---- x0 = blend(attn) + residual; x1 = LN1(x0) ----
                    for tl in range(2):
                        sblk = j * 2 + tl
                        rsb = stg.tile([128, D], BF16, tag="rsb", bufs=4)
                        dma_q.dma_start(
                            rsb[:], src[base + tl * 128:base + tl * 128 + 128, :])
                        xsb = stg.tile([128, D], F32, tag="stg")
                        dma_q.dma_start(
                            xsb[:], xs_d[sblk * 128:sblk * 128 + 128, :])
                        t0 = stg.tile([128, D], F32, tag="stg")
                        nc.vector.scalar_tensor_tensor(
                            t0[:], rsb[:], q1m[:, sblk:sblk + 1], xsb[:],
                            ALU.mult, ALU.add)
                        x0 = stg.tile([128, D], F32, tag="stg")
                        nc.vector.scalar_tensor_tensor(
                            x0[:], vtb[:], qp[:, sblk:sblk + 1], t0[:],
                            ALU.mult, ALU.add)
                        x1 = x1p.tile([128, D], F32, tag="x1")
                        layer_norm(x0[:], ln1w[:], ln1b[:], x1[:])
                        x1s.append(x1)

                # ---- x1^T (bf16) for the pair ----
                x1t = x1tp.tile([128, DC * TPP], BF16, tag="x1t")
                for q in range(BPP):
                    for c in range(DC):
                        tp = pmed.tile([128, 128], F32, tag="med")
                        nc.tensor.transpose(
                            tp[:], x1s[q][:, c * 128:c * 128 + 128], ident[:])
                        nc.scalar.copy(
                            x1t[:, c * TPP + q * 128:c * TPP + q * 128 + 128],
                            tp[:])

                # ---- hT = relu(W1^T x1^T + b1) (bf16) ----
                hts = htp.tile([128, FB * TPP], BF16, tag="hts")
                for fb in range(FB):
                    w1s = w1p.tile([128, D], BF16, tag="w1s")
                    dma_q.dma_start(w1s[:], w1_d[fb])
                    hps = pmed.tile([128, TPP], F32, tag="med")
                    for c in range(DC):
                        nc.tensor.matmul(hps[:], w1s[:, c * 128:c * 128 + 128],
                                         x1t[:, c * TPP:(c + 1) * TPP],
                                         start=(c == 0), stop=(c == DC - 1))
                    nc.scalar.activation(hts[:, fb * TPP:(fb + 1) * TPP],
                                         hps[:], AF.Relu,
                                         bias=b1t[:, fb:fb + 1], scale=1.0)

                # ---- y = hT.T @ W2 accumulated over fb groups ----
                ys_prev = [None] * BPP
                NG = FB // GF
                w2cache = {}
                for g in range(NG):
                    for q in range(BPP):
                        yps = pbig.tile([128, D], F32, tag="big")
                        for ji in range(GF):
                            fb = g * GF + ji
                            if q == 0:
                                w2s = w2p.tile([128, D], BF16, tag="w2s")
                                dma_q.dma_start(
                                    w2s[:], w2_d[fb * 128:fb * 128 + 128, :])
                                w2cache[fb] = w2s
                            w2s = w2cache[fb]
                            for ec in range(EC):
                                nc.tensor.matmul(
                                    yps[:, ec * 512:ec * 512 + 512],
                                    hts[:, fb * TPP + q * 128:
                                        fb * TPP + q * 128 + 128],
                                    w2s[:, ec * 512:ec * 512 + 512],
                                    start=(ji == 0), stop=(ji == GF - 1))
                        ysn = ysp.tile([128, D], F32, tag="ysb")
                        if g == 0:
                            nc.scalar.copy(ysn[:], yps[:])
                        else:
                            nc.vector.scalar_tensor_tensor(
                                ysn[:], yps[:], 1.0, ys_prev[q][:],
                                ALU.mult, ALU.add)
                        ys_prev[q] = ysn

                # ---- x2 = x1 + y + b2; out = LN2(x2) ----
                for q in range(BPP):
                    x2 = outp.tile([128, D], F32, tag="x2")
                    nc.vector.scalar_tensor_tensor(
                        x2[:], ys_prev[q][:], 1.0, x1s[q][:],
                        ALU.mult, ALU.add)
                    nc.vector.tensor_add(x2[:], x2[:], b2b[:])
                    ot = outp.tile([128, D], F32, tag="ot")
                    layer_norm(x2[:], ln2w[:], ln2b[:], ot[:])
                    row = (p * BPP + q) * 128
                    nc.sync.dma_start(out_d[row:row + 128, :], ot[:])

    nc.compile()
    return nc


# ------------------------- host side -------------------------

_NC_CACHE = {}


def _get_nc(cfg_key):
    if cfg_key not in _NC_CACHE:
        _NC_CACHE[cfg_key] = build_nc(**CFG_MAIN)
    return _NC_CACHE[cfg_key]


def core_chunks(c, B, S, ncores):
    """(batch, c0) chunk list owned by core c: 3 phase-A chunks + 1 phase-B."""
    CPB = S // 256
    out = [((3 * c + k) // CPB, (3 * c + k) % CPB) for k in range(3)]
    out.append((B - 1, c))
    return out


def make_in_maps(inputs, B, S, D, FF, ncores, HPC):
    """Build the per-core input dicts from the full (unsharded) inputs."""
    TB = S // 128
    shard = B * S // ncores
    SB = shard // 128
    FB = FF // 128
    H = ncores * HPC
    bf = ml_dtypes.bfloat16

    x = np.ascontiguousarray(
        np.asarray(inputs["input"], dtype=np.float32).reshape(B * S, D))
    xb = x.astype(bf)
    pad = np.asarray(inputs["padding_mask"], dtype=bool)
    Wq = np.asarray(inputs["Wq"], dtype=np.float32) * 0.125
    Wk = np.asarray(inputs["Wk"], dtype=np.float32)
    Wv = np.asarray(inputs["Wv"], dtype=np.float32)
    bq = np.asarray(inputs["bq"], dtype=np.float32) * 0.125
    bk = np.asarray(inputs["bk"], dtype=np.float32)
    bvv = np.asarray(inputs["bv"], dtype=np.float32)

    padb = np.where(pad, np.float32(NEG), np.float32(0.0))
    padb = np.ascontiguousarray(
        padb.reshape(B, TB, 128).transpose(0, 2, 1))

    cmask = np.zeros((128, 128), dtype=np.float32)
    cmask[np.tril_indices(128, -1)] = NEG

    w1 = np.asarray(inputs["ff1_w"], dtype=np.float32)
    w1s = np.ascontiguousarray(
        w1.reshape(D // 128, 128, FB, 128).transpose(2, 1, 0, 3)
        .reshape(FB, 128, D)).astype(bf)
    w2 = np.asarray(inputs["ff2_w"], dtype=np.float32).astype(bf)
    b1 = np.asarray(inputs["ff1_b"], dtype=np.float32)
    b1t = np.ascontiguousarray(b1.reshape(FB, 128).T)
    b2b = np.ascontiguousarray(
        np.broadcast_to(np.asarray(inputs["ff2_b"], np.float32), (128, D)))

    def bc(name):
        return np.ascontiguousarray(np.broadcast_to(
            np.asarray(inputs[name], np.float32), (128, D)))

    ident = np.eye(128, dtype=np.float32)
    padflat = pad.reshape(B * S)

    # softmax rows sum to 1, so every head's +bv contributes exactly bv_h to
    # the 16-head sum; fold sum(bv)/H into the residual (pad terms cancel).
    bvall = bvv.sum(axis=0).astype(np.float32) / H

    in_maps = []
    for c in range(ncores):
        h0 = c * HPC
        tok_idx = np.concatenate([
            bb * S + c0 * 256 + np.arange(256)
            for bb, c0 in core_chunks(c, B, S, ncores)])
        prow = padflat[tok_idx].reshape(SB, 128).T
        prow = prow.astype(np.float32)
        m = {
            "xb": xb,
            "xs": np.ascontiguousarray(x[tok_idx] + bvall[None, :]),
            "wq": np.ascontiguousarray(Wq[h0:h0 + HPC]).astype(bf),
            "wk": np.ascontiguousarray(Wk[h0:h0 + HPC]).astype(bf),
            "wv": np.ascontiguousarray(Wv[h0:h0 + HPC]).astype(bf),
            "bqkc": np.ascontiguousarray(np.stack(
                [np.concatenate([bq[h0], bq[h0 + 1]]),
                 np.concatenate([bk[h0], bk[h0 + 1]])], axis=1)
            ).astype(np.float32),
            "padb": padb,
            "q1m": np.ascontiguousarray((1.0 - prow) / H),
            "qp": np.ascontiguousarray(prow / H),
            "cmask": cmask,
            "ident": ident,
            "w1s": w1s,
            "w2": w2,
            "b1t": b1t,
            "b2b": b2b,
            "ln1w": bc("ln1_w"),
            "ln1b": bc("ln1_b"),
            "ln2w": bc("ln2_w"),
            "ln2b": bc("ln2_b"),
        }
        in_maps.append(m)
    return in_maps


def assemble_out(results, B, S, D, ncores):
    out = np.empty((B * S, D), dtype=np.float32)
    for c in range(ncores):
        r_ = np.asarray(results[c]["out"])
        for k, (bb, c0) in enumerate(core_chunks(c, B, S, ncores)):
            out[bb * S + c0 * 256: bb * S + c0 * 256 + 256] = \
                r_[k * 256:(k + 1) * 256]
    return out.reshape(B, S, D)


def kernel(**inputs):
    from concourse.bass_utils import run_bass_kernel_spmd
    cfg = CFG_MAIN
    B, S, D = cfg["B"], cfg["S"], cfg["D"]
    ncores = cfg["ncores"]
    nc = _get_nc("main")
    in_maps = make_in_maps(inputs, **cfg)
    res = run_bass_kernel_spmd(nc, in_maps, list(range(ncores)))
    return assemble_out(res.results, B, S, D, ncores).astype(np.float32)
